# revision 17
# baseline (speedup 1.0000x reference)
"""
Trainium2 Bass kernel for nn_DenseFeatureNumericEmbedding (v2).

Computes, per feature f (F=128 independent tiny MLPs):
    h[b,f,:]   = relu(x[b,f] * w1[f,:] + b1[f,:])            # [B, F, H]
    out[b,f,:] = h[b,f,:] @ w2[f,:,:] + b2[f,:]              # [B, F, E]
    returns out.reshape(B, F*E)                              # [16384, 4096] fp32

Sharding: data-parallel over batch across 8 NeuronCores (2048 rows/core),
params replicated. No collectives.

v2 design (vs v1 baseline at 512us):
  - Device stores outT [F*E, BL] in fp16; the HOST transposes to [BL, F*E],
    adds b2, and casts to fp32.  This removes all 512 PE transposes, their
    LDWEIGHTS, the DVE staging copies, and halves output DMA bytes.
  - L1 matmuls are zero-padded to K=32 (stationary rows 2..32 of each row
    group are zero, xq rows 2..32 of each group are memset to zero) so the
    PE array shows high activity and the HAM clock gate stays at 2.4 GHz.
  - PSUM carving: pre pool [128,1024] fp32 x2 bufs (4 banks) for L1 output,
    pout pool [128,1024] fp32 x2 bufs (4 banks) for two quads of L2 output.
    Relu runs at FD=1024 split ACT/DVE; output copy (pure fp32->fp16 copy,
    bias folded out to host) runs at FD=1024.
  - Output DMA batched per 4 quads: 32 DMAs of 512KB, 1KB runs.

Per-core dataflow (per 512-batch chunk, per quad of 4 features):
  L1   TensorE: 4 row-tiled K=32 matmuls -> pre_a/pre_b [128, 1024] fp32.
  RELU ScalarE activation(Relu) / VectorE tensor_scalar_max(0) split,
       PSUM -> SBUF bf16 hT [128, 2048].
  L2   TensorE: 4 col-tiled K=128 matmuls -> pout2 [128, 512] slice.
  COPY fp32 PSUM -> fp16 SBUF staging (DVE tensor_copy / ACT Copy).
  DMA  outT [F*E, BL] fp16, 1KB contiguous runs.
"""

import sys

sys.path.insert(0, "/opt/trn_rl_repo")

import numpy as np
import ml_dtypes

import concourse.bass as bass
import concourse.tile as tile
from concourse import bacc, mybir
from concourse.bass_utils import run_bass_kernel_spmd

BF16 = ml_dtypes.bfloat16
FP16 = np.float16

B = 16384
F = 128
H = 128
E = 32
NCORES = 8
BL = B // NCORES          # 2048 rows per core
CHUNK = 512               # batch columns per inner tile (1 PSUM bank fp32)
NCHUNK = BL // CHUNK      # 4
NQUAD = F // 4            # 32 quads of 4 features

CONFIG = {
    "RELU_ACT_OF_16": 11,  # of every 16 relu instrs, this many on ScalarE
    "OUT_ACT_OF_16": 0,    # of every 16 out-copies, this many on ScalarE
    "VARIANT_ID": 0,       # busts the NEFF cache between variants
}

_COMPILED = None


def _build_bass():
    nc = bacc.Bacc("TRN2", target_bir_lowering=False, debug=False,
                   num_devices=NCORES)
    dt = mybir.dt

    xt2 = nc.dram_tensor("xt2", [2 * F, BL], dt.bfloat16, kind="ExternalInput").ap()
    w1b1q = nc.dram_tensor("w1b1q", [128, F * H], dt.bfloat16, kind="ExternalInput").ap()
    w2s = nc.dram_tensor("w2s", [H, F * E], dt.bfloat16, kind="ExternalInput").ap()
    outT = nc.dram_tensor("outT", [F * E, BL], dt.bfloat16, kind="ExternalOutput").ap()

    # DRAM views
    # xt2 rows: 2f + r  (f feature, r 0=x / 1=ones); g = 2j + r below
    xt2_r = xt2.rearrange("(q g) n -> g q n", g=8)        # [8, NQUAD, BL]
    outT_r = outT.rearrange("(q p) n -> q p n", p=128)    # [NQUAD, 128, BL]

    for _ in range(CONFIG["VARIANT_ID"]):
        nc.sync.nop()

    relu_act, out_act = CONFIG["RELU_ACT_OF_16"], CONFIG["OUT_ACT_OF_16"]

    with tile.TileContext(nc) as tc:
        with (
            tc.tile_pool(name="params", bufs=1) as params,
            tc.tile_pool(name="h", bufs=4) as h_pool,
            tc.tile_pool(name="outs", bufs=3) as outs_pool,
            tc.tile_pool(name="pre", bufs=3, space="PSUM") as pre_pool,
            tc.tile_pool(name="pout", bufs=2, space="PSUM") as pout_pool,
        ):
            # Two persistent xq buffers (even/odd chunks).  Rows 32j+0/1 of
            # each row group hold (x, ones) per feature.
            xqs = [params.tile([128, NQUAD * CHUNK], dt.bfloat16,
                               tag=f"xq{i}", name=f"xq{i}")
                   for i in range(2)]

            # chunk-0 x data ahead of the params on the sync queue; quads
            # 0-1 split into tiny leading DMAs so the first L1 matmuls only
            # wait for ~4KB + the first w1b1 slice, not the whole chunk.
            xq0 = xqs[0]
            for j in range(4):
                nc.sync.dma_start(
                    out=xq0[32 * j:32 * j + 2, :2 * CHUNK].rearrange(
                        "r (q n) -> r q n", n=CHUNK),
                    in_=xt2_r[2 * j:2 * j + 2, :2, bass.ts(0, CHUNK)],
                )
            w1b1q_sb = params.tile([128, F * H], dt.bfloat16, tag="w1b1q")
            nc.sync.dma_start(out=w1b1q_sb[:, :2 * H],
                              in_=w1b1q[:, :2 * H])
            for j in range(4):
                nc.sync.dma_start(
                    out=xq0[32 * j:32 * j + 2, 2 * CHUNK:].rearrange(
                        "r (q n) -> r q n", n=CHUNK),
                    in_=xt2_r[2 * j:2 * j + 2, 2:, bass.ts(0, CHUNK)],
                )
            for p in range(1, 8):
                nc.sync.dma_start(out=w1b1q_sb[:, bass.ts(p, F * H // 8)],
                                  in_=w1b1q[:, bass.ts(p, F * H // 8)])
            nc.sync.dma_start(out=w1b1q_sb[:, 2 * H:F * H // 8],
                              in_=w1b1q[:, 2 * H:F * H // 8])
            w2_sb = params.tile([H, F * E], dt.bfloat16, tag="w2s")
            nc.sync.dma_start(out=w2_sb[:], in_=w2s[:])

            relu_credit = 0
            out_credit = 0
            for c in range(NCHUNK):
                xq = xqs[c % 2]
                # xq[32j + r, 512q + cc] = xt2[8q + 2j + r, 512c + cc]
                # (chunk 0's DMAs were issued before the params above)
                if c > 0:
                    for j in range(4):
                        nc.sync.dma_start(
                            out=xq[32 * j:32 * j + 2, :].rearrange(
                                "r (q n) -> r q n", n=CHUNK),
                            in_=xt2_r[2 * j:2 * j + 2, :, bass.ts(c, CHUNK)],
                        )

                hT_prev = None
                for q in range(NQUAD + 1):
                    if q < NQUAD:
                        # ---- L1: 4 features, row groups 0..3, K=2 ----
                        pre_a = pre_pool.tile([128, 2 * CHUNK], dt.float32,
                                              tag="pre")
                        pre_b = pre_pool.tile([128, 2 * CHUNK], dt.float32,
                                              tag="pre")
                        for j in range(4):
                            tgt = pre_a if j < 2 else pre_b
                            nc.tensor.matmul(
                                tgt[:, bass.ts(j % 2, CHUNK)],
                                lhsT=w1b1q_sb[32 * j:32 * j + 2,
                                              bass.ts(q, H)],
                                rhs=xq[32 * j:32 * j + 2, bass.ts(q, CHUNK)],
                                start=True, stop=True,
                                tile_position=(32 * j, 0),
                            )

                        # ---- relu + cast bf16, split ACT / DVE ----
                        hT = h_pool.tile([128, 4 * CHUNK], dt.bfloat16,
                                         tag="h")
                        for half, hsrc in ((0, pre_a), (1, pre_b)):
                            dst = hT[:, bass.ts(half, 2 * CHUNK)]
                            relu_credit += relu_act
                            if relu_credit >= 16:
                                relu_credit -= 16
                                nc.scalar.activation(
                                    dst, hsrc[:],
                                    mybir.ActivationFunctionType.Relu)
                            else:
                                nc.vector.tensor_scalar_max(dst, hsrc[:],
                                                            0.0)
                    if hT_prev is None:
                        hT_prev = hT
                        continue

                    # ---- L2 for the PREVIOUS quad, software-pipelined so
                    # the PE has ready matmuls while this quad's relu runs --
                    qq = q - 1
                    pout = pout_pool.tile([128, CHUNK], dt.float32,
                                          tag="pout")
                    for j in range(4):
                        f = 4 * qq + j
                        nc.tensor.matmul(
                            pout[32 * j:32 * j + 32, :],
                            lhsT=w2_sb[:, bass.ts(f, E)],
                            rhs=hT_prev[:, bass.ts(j, CHUNK)],
                            start=True, stop=True,
                            tile_position=(0, 32 * j),
                        )
                    hT_prev = hT if q < NQUAD else None

                    # ---- copy quad PSUM fp32 -> SBUF bf16 ----
                    if qq % 4 == 0:
                        outTs = outs_pool.tile([128, 4 * CHUNK], dt.bfloat16,
                                               tag="outs")
                    dst = outTs[:, bass.ts(qq % 4, CHUNK)]
                    out_credit += out_act
                    if out_credit >= 16:
                        out_credit -= 16
                        nc.scalar.add(dst, pout[:], 0.0)
                    else:
                        nc.vector.tensor_scalar_add(dst, pout[:], 0.0)

                    # ---- store 4 quads: outT rows 128(qq-3)..128(qq+1) ----
                    # (gpsimd DMA queue, so the serial sync queue stays free
                    # for the next chunk's xq prefetch)
                    if qq % 4 == 3:
                        # SBUF src must keep the partition dim outermost;
                        # permute the DRAM view instead.
                        nc.gpsimd.dma_start(
                            out=outT_r[qq - 3:qq + 1, :, bass.ts(c, CHUNK)
                                       ].rearrange("q p n -> p q n"),
                            in_=outTs[:].rearrange("p (k n) -> p k n",
                                                   n=CHUNK),
                        )

    nc.compile()
    return nc


def _prep_inputs(x, w1, b1, w2, b2):
    """Host-side packing of parameters + per-core x shards."""
    w1b1q = np.zeros((128, F * H), dtype=BF16)
    for f in range(F):
        q, j = divmod(f, 4)
        w1b1q[32 * j + 0, H * q:H * q + H] = w1[f].astype(BF16)
        w1b1q[32 * j + 1, H * q:H * q + H] = b1[f].astype(BF16)

    w2s = np.ascontiguousarray(
        w2.transpose(1, 0, 2).reshape(H, F * E)).astype(BF16)

    in_maps = []
    for core in range(NCORES):
        xs = x[core * BL:(core + 1) * BL]          # [BL, F]
        xt2 = np.empty((2 * F, BL), dtype=BF16)
        xt2[0::2] = xs.T.astype(BF16)
        xt2[1::2] = BF16(1.0)
        in_maps.append({"xt2": xt2, "w1b1q": w1b1q, "w2s": w2s})
    return in_maps


def _get_compiled():
    global _COMPILED
    if _COMPILED is None:
        _COMPILED = _build_bass()
    return _COMPILED


def reset_compiled():
    global _COMPILED
    _COMPILED = None


def kernel(x, w1, b1, w2, b2, _trace=False, _trace_kwargs=None):
    nc = _get_compiled()
    x = np.asarray(x, dtype=np.float32)
    w2 = np.asarray(w2, dtype=np.float32)
    b2 = np.asarray(b2, dtype=np.float32)
    in_maps = _prep_inputs(
        x, np.asarray(w1, dtype=np.float32),
        np.asarray(b1, dtype=np.float32), w2, b2)
    res = run_bass_kernel_spmd(
        nc, in_maps, core_ids=list(range(NCORES)),
        trace=_trace, **(_trace_kwargs or {}))
    b2f = b2.reshape(F * E).astype(np.float32)          # fe = f*E + e
    shards = []
    for i in range(NCORES):
        oT = np.asarray(res.results[i]["outT"])          # [F*E, BL] bf16
        shards.append((oT.astype(np.float32) + b2f[:, None]).T)
    full = np.ascontiguousarray(np.concatenate(shards, axis=0),
                                dtype=np.float32)
    if _trace:
        return full, res
    return full


if __name__ == "__main__":
    rng = np.random.default_rng(0)
    x = rng.standard_normal((B, F), dtype=np.float32)
    w1 = rng.standard_normal((F, H), dtype=np.float32)
    b1 = rng.standard_normal((F, H), dtype=np.float32)
    w2 = (rng.standard_normal((F, H, E), dtype=np.float32) / np.sqrt(H)).astype(np.float32)
    b2 = rng.standard_normal((F, E), dtype=np.float32) / np.sqrt(H)
    got = kernel(x=x, w1=w1, b1=b1, w2=w2, b2=b2)
    h = np.maximum(x[:, :, None] * w1[None] + b1[None], 0.0)
    want = (np.einsum("bfh,fhe->bfe", h, w2) + b2[None]).reshape(B, F * E)
    err = np.abs(got - want).max() / np.abs(want).max()
    print("self-test scale-relative max err:", err)


# revision 18
# speedup vs baseline: 1.0171x; 1.0171x over previous
"""
Trainium2 Bass kernel for nn_DenseFeatureNumericEmbedding (v2).

Computes, per feature f (F=128 independent tiny MLPs):
    h[b,f,:]   = relu(x[b,f] * w1[f,:] + b1[f,:])            # [B, F, H]
    out[b,f,:] = h[b,f,:] @ w2[f,:,:] + b2[f,:]              # [B, F, E]
    returns out.reshape(B, F*E)                              # [16384, 4096] fp32

Sharding: data-parallel over batch across 8 NeuronCores (2048 rows/core),
params replicated. No collectives.

v2 design (vs v1 baseline at 512us):
  - Device stores outT [F*E, BL] in fp16; the HOST transposes to [BL, F*E],
    adds b2, and casts to fp32.  This removes all 512 PE transposes, their
    LDWEIGHTS, the DVE staging copies, and halves output DMA bytes.
  - L1 matmuls are zero-padded to K=32 (stationary rows 2..32 of each row
    group are zero, xq rows 2..32 of each group are memset to zero) so the
    PE array shows high activity and the HAM clock gate stays at 2.4 GHz.
  - PSUM carving: pre pool [128,1024] fp32 x2 bufs (4 banks) for L1 output,
    pout pool [128,1024] fp32 x2 bufs (4 banks) for two quads of L2 output.
    Relu runs at FD=1024 split ACT/DVE; output copy (pure fp32->fp16 copy,
    bias folded out to host) runs at FD=1024.
  - Output DMA batched per 4 quads: 32 DMAs of 512KB, 1KB runs.

Per-core dataflow (per 512-batch chunk, per quad of 4 features):
  L1   TensorE: 4 row-tiled K=32 matmuls -> pre_a/pre_b [128, 1024] fp32.
  RELU ScalarE activation(Relu) / VectorE tensor_scalar_max(0) split,
       PSUM -> SBUF bf16 hT [128, 2048].
  L2   TensorE: 4 col-tiled K=128 matmuls -> pout2 [128, 512] slice.
  COPY fp32 PSUM -> fp16 SBUF staging (DVE tensor_copy / ACT Copy).
  DMA  outT [F*E, BL] fp16, 1KB contiguous runs.
"""

import sys

sys.path.insert(0, "/opt/trn_rl_repo")

import numpy as np
import ml_dtypes

import concourse.bass as bass
import concourse.tile as tile
from concourse import bacc, mybir
from concourse.bass_utils import run_bass_kernel_spmd

BF16 = ml_dtypes.bfloat16
FP16 = np.float16

B = 16384
F = 128
H = 128
E = 32
NCORES = 8
BL = B // NCORES          # 2048 rows per core
CHUNK = 512               # batch columns per inner tile (1 PSUM bank fp32)
NCHUNK = BL // CHUNK      # 4
NQUAD = F // 4            # 32 quads of 4 features

CONFIG = {
    "RELU_ACT_OF_16": 11,  # of every 16 relu instrs, this many on ScalarE
    "OUT_ACT_OF_16": 0,    # of every 16 out-copies, this many on ScalarE
    "VARIANT_ID": 0,       # busts the NEFF cache between variants
}

_COMPILED = None


def _build_bass():
    nc = bacc.Bacc("TRN2", target_bir_lowering=False, debug=False,
                   num_devices=NCORES)
    dt = mybir.dt

    xt2 = nc.dram_tensor("xt2", [2 * F, BL], dt.bfloat16, kind="ExternalInput").ap()
    w1b1q = nc.dram_tensor("w1b1q", [128, F * H], dt.bfloat16, kind="ExternalInput").ap()
    w2s = nc.dram_tensor("w2s", [H, F * E], dt.bfloat16, kind="ExternalInput").ap()
    outT = nc.dram_tensor("outT", [F * E, BL], dt.bfloat16, kind="ExternalOutput").ap()

    # DRAM views
    # xt2 rows: 2f + r  (f feature, r 0=x / 1=ones); g = 2j + r below
    xt2_r = xt2.rearrange("(q g) n -> g q n", g=8)        # [8, NQUAD, BL]
    outT_r = outT.rearrange("(q p) n -> q p n", p=128)    # [NQUAD, 128, BL]

    for _ in range(CONFIG["VARIANT_ID"]):
        nc.sync.nop()

    relu_act, out_act = CONFIG["RELU_ACT_OF_16"], CONFIG["OUT_ACT_OF_16"]

    with tile.TileContext(nc) as tc:
        with (
            tc.tile_pool(name="params", bufs=1) as params,
            tc.tile_pool(name="h", bufs=4) as h_pool,
            tc.tile_pool(name="outs", bufs=3) as outs_pool,
            tc.tile_pool(name="pre", bufs=3, space="PSUM") as pre_pool,
            tc.tile_pool(name="pout", bufs=2, space="PSUM") as pout_pool,
        ):
            # Two persistent xq buffers (even/odd chunks).  Rows 32j+0/1 of
            # each row group hold (x, ones) per feature.
            xqs = [params.tile([128, NQUAD * CHUNK], dt.bfloat16,
                               tag=f"xq{i}", name=f"xq{i}")
                   for i in range(2)]

            # chunk-0 x data ahead of the params on the sync queue
            xq0 = xqs[0]
            for j in range(4):
                nc.sync.dma_start(
                    out=xq0[32 * j:32 * j + 2, :].rearrange(
                        "r (q n) -> r q n", n=CHUNK),
                    in_=xt2_r[2 * j:2 * j + 2, :, bass.ts(0, CHUNK)],
                )

            w1b1q_sb = params.tile([128, F * H], dt.bfloat16, tag="w1b1q")
            for p in range(4):
                nc.sync.dma_start(out=w1b1q_sb[:, bass.ts(p, F * H // 4)],
                                  in_=w1b1q[:, bass.ts(p, F * H // 4)])
            w2_sb = params.tile([H, F * E], dt.bfloat16, tag="w2s")
            nc.sync.dma_start(out=w2_sb[:], in_=w2s[:])

            relu_credit = 0
            out_credit = 0
            for c in range(NCHUNK):
                xq = xqs[c % 2]
                # xq[32j + r, 512q + cc] = xt2[8q + 2j + r, 512c + cc]
                # (chunk 0's DMAs were issued before the params above)
                if c > 0:
                    for j in range(4):
                        nc.sync.dma_start(
                            out=xq[32 * j:32 * j + 2, :].rearrange(
                                "r (q n) -> r q n", n=CHUNK),
                            in_=xt2_r[2 * j:2 * j + 2, :, bass.ts(c, CHUNK)],
                        )

                hT_prev = None
                for q in range(NQUAD + 1):
                    if q < NQUAD:
                        # ---- L1: 4 features, row groups 0..3, K=2 ----
                        pre_a = pre_pool.tile([128, 2 * CHUNK], dt.float32,
                                              tag="pre")
                        pre_b = pre_pool.tile([128, 2 * CHUNK], dt.float32,
                                              tag="pre")
                        for j in range(4):
                            tgt = pre_a if j < 2 else pre_b
                            nc.tensor.matmul(
                                tgt[:, bass.ts(j % 2, CHUNK)],
                                lhsT=w1b1q_sb[32 * j:32 * j + 2,
                                              bass.ts(q, H)],
                                rhs=xq[32 * j:32 * j + 2, bass.ts(q, CHUNK)],
                                start=True, stop=True,
                                tile_position=(32 * j, 0),
                            )

                        # ---- relu + cast bf16, split ACT / DVE ----
                        hT = h_pool.tile([128, 4 * CHUNK], dt.bfloat16,
                                         tag="h")
                        for half, hsrc in ((0, pre_a), (1, pre_b)):
                            dst = hT[:, bass.ts(half, 2 * CHUNK)]
                            relu_credit += relu_act
                            if relu_credit >= 16:
                                relu_credit -= 16
                                nc.scalar.activation(
                                    dst, hsrc[:],
                                    mybir.ActivationFunctionType.Relu)
                            else:
                                nc.vector.tensor_scalar_max(dst, hsrc[:],
                                                            0.0)
                    if hT_prev is None:
                        hT_prev = hT
                        continue

                    # ---- L2 for the PREVIOUS quad, software-pipelined so
                    # the PE has ready matmuls while this quad's relu runs --
                    qq = q - 1
                    pout = pout_pool.tile([128, CHUNK], dt.float32,
                                          tag="pout")
                    for j in range(4):
                        f = 4 * qq + j
                        nc.tensor.matmul(
                            pout[32 * j:32 * j + 32, :],
                            lhsT=w2_sb[:, bass.ts(f, E)],
                            rhs=hT_prev[:, bass.ts(j, CHUNK)],
                            start=True, stop=True,
                            tile_position=(0, 32 * j),
                        )
                    hT_prev = hT if q < NQUAD else None

                    # ---- copy quad PSUM fp32 -> SBUF bf16 ----
                    if qq % 4 == 0:
                        outTs = outs_pool.tile([128, 4 * CHUNK], dt.bfloat16,
                                               tag="outs")
                    dst = outTs[:, bass.ts(qq % 4, CHUNK)]
                    out_credit += out_act
                    if out_credit >= 16:
                        out_credit -= 16
                        nc.scalar.add(dst, pout[:], 0.0)
                    else:
                        nc.vector.tensor_scalar_add(dst, pout[:], 0.0)

                    # ---- store 4 quads: outT rows 128(qq-3)..128(qq+1) ----
                    # (gpsimd DMA queue, so the serial sync queue stays free
                    # for the next chunk's xq prefetch)
                    if qq % 4 == 3:
                        # SBUF src must keep the partition dim outermost;
                        # permute the DRAM view instead.
                        nc.gpsimd.dma_start(
                            out=outT_r[qq - 3:qq + 1, :, bass.ts(c, CHUNK)
                                       ].rearrange("q p n -> p q n"),
                            in_=outTs[:].rearrange("p (k n) -> p k n",
                                                   n=CHUNK),
                        )

    nc.compile()
    return nc


def _prep_inputs(x, w1, b1, w2, b2):
    """Host-side packing of parameters + per-core x shards."""
    w1b1q = np.zeros((128, F * H), dtype=BF16)
    for f in range(F):
        q, j = divmod(f, 4)
        w1b1q[32 * j + 0, H * q:H * q + H] = w1[f].astype(BF16)
        w1b1q[32 * j + 1, H * q:H * q + H] = b1[f].astype(BF16)

    w2s = np.ascontiguousarray(
        w2.transpose(1, 0, 2).reshape(H, F * E)).astype(BF16)

    in_maps = []
    for core in range(NCORES):
        xs = x[core * BL:(core + 1) * BL]          # [BL, F]
        xt2 = np.empty((2 * F, BL), dtype=BF16)
        xt2[0::2] = xs.T.astype(BF16)
        xt2[1::2] = BF16(1.0)
        in_maps.append({"xt2": xt2, "w1b1q": w1b1q, "w2s": w2s})
    return in_maps


def _get_compiled():
    global _COMPILED
    if _COMPILED is None:
        _COMPILED = _build_bass()
    return _COMPILED


def reset_compiled():
    global _COMPILED
    _COMPILED = None


def kernel(x, w1, b1, w2, b2, _trace=False, _trace_kwargs=None):
    nc = _get_compiled()
    x = np.asarray(x, dtype=np.float32)
    w2 = np.asarray(w2, dtype=np.float32)
    b2 = np.asarray(b2, dtype=np.float32)
    in_maps = _prep_inputs(
        x, np.asarray(w1, dtype=np.float32),
        np.asarray(b1, dtype=np.float32), w2, b2)
    res = run_bass_kernel_spmd(
        nc, in_maps, core_ids=list(range(NCORES)),
        trace=_trace, **(_trace_kwargs or {}))
    b2f = b2.reshape(F * E).astype(np.float32)          # fe = f*E + e
    shards = []
    for i in range(NCORES):
        oT = np.asarray(res.results[i]["outT"])          # [F*E, BL] bf16
        shards.append((oT.astype(np.float32) + b2f[:, None]).T)
    full = np.ascontiguousarray(np.concatenate(shards, axis=0),
                                dtype=np.float32)
    if _trace:
        return full, res
    return full


if __name__ == "__main__":
    rng = np.random.default_rng(0)
    x = rng.standard_normal((B, F), dtype=np.float32)
    w1 = rng.standard_normal((F, H), dtype=np.float32)
    b1 = rng.standard_normal((F, H), dtype=np.float32)
    w2 = (rng.standard_normal((F, H, E), dtype=np.float32) / np.sqrt(H)).astype(np.float32)
    b2 = rng.standard_normal((F, E), dtype=np.float32) / np.sqrt(H)
    got = kernel(x=x, w1=w1, b1=b1, w2=w2, b2=b2)
    h = np.maximum(x[:, :, None] * w1[None] + b1[None], 0.0)
    want = (np.einsum("bfh,fhe->bfe", h, w2) + b2[None]).reshape(B, F * E)
    err = np.abs(got - want).max() / np.abs(want).max()
    print("self-test scale-relative max err:", err)


# revision 22
# speedup vs baseline: 1.0300x; 1.0127x over previous
"""
Trainium2 Bass kernel for nn_DenseFeatureNumericEmbedding (v2).

Computes, per feature f (F=128 independent tiny MLPs):
    h[b,f,:]   = relu(x[b,f] * w1[f,:] + b1[f,:])            # [B, F, H]
    out[b,f,:] = h[b,f,:] @ w2[f,:,:] + b2[f,:]              # [B, F, E]
    returns out.reshape(B, F*E)                              # [16384, 4096] fp32

Sharding: data-parallel over batch across 8 NeuronCores (2048 rows/core),
params replicated. No collectives.

v2 design (vs v1 baseline at 512us):
  - Device stores outT [F*E, BL] in fp16; the HOST transposes to [BL, F*E],
    adds b2, and casts to fp32.  This removes all 512 PE transposes, their
    LDWEIGHTS, the DVE staging copies, and halves output DMA bytes.
  - L1 matmuls are zero-padded to K=32 (stationary rows 2..32 of each row
    group are zero, xq rows 2..32 of each group are memset to zero) so the
    PE array shows high activity and the HAM clock gate stays at 2.4 GHz.
  - PSUM carving: pre pool [128,1024] fp32 x2 bufs (4 banks) for L1 output,
    pout pool [128,1024] fp32 x2 bufs (4 banks) for two quads of L2 output.
    Relu runs at FD=1024 split ACT/DVE; output copy (pure fp32->fp16 copy,
    bias folded out to host) runs at FD=1024.
  - Output DMA batched per 4 quads: 32 DMAs of 512KB, 1KB runs.

Per-core dataflow (per 512-batch chunk, per quad of 4 features):
  L1   TensorE: 4 row-tiled K=32 matmuls -> pre_a/pre_b [128, 1024] fp32.
  RELU ScalarE activation(Relu) / VectorE tensor_scalar_max(0) split,
       PSUM -> SBUF bf16 hT [128, 2048].
  L2   TensorE: 4 col-tiled K=128 matmuls -> pout2 [128, 512] slice.
  COPY fp32 PSUM -> fp16 SBUF staging (DVE tensor_copy / ACT Copy).
  DMA  outT [F*E, BL] fp16, 1KB contiguous runs.
"""

import sys

sys.path.insert(0, "/opt/trn_rl_repo")

import numpy as np
import ml_dtypes

import concourse.bass as bass
import concourse.tile as tile
from concourse import bacc, mybir
from concourse.bass_utils import run_bass_kernel_spmd

BF16 = ml_dtypes.bfloat16
FP16 = np.float16

B = 16384
F = 128
H = 128
E = 32
NCORES = 8
BL = B // NCORES          # 2048 rows per core
CHUNK = 512               # batch columns per inner tile (1 PSUM bank fp32)
NCHUNK = BL // CHUNK      # 4
NQUAD = F // 4            # 32 quads of 4 features

CONFIG = {
    "RELU_ACT_OF_16": 11,  # of every 16 relu instrs, this many on ScalarE
    "OUT_ACT_OF_16": 0,    # of every 16 out-copies, this many on ScalarE
    "VARIANT_ID": 0,       # busts the NEFF cache between variants
}

_COMPILED = None


def _build_bass():
    nc = bacc.Bacc("TRN2", target_bir_lowering=False, debug=False,
                   num_devices=NCORES)
    dt = mybir.dt

    xt2 = nc.dram_tensor("xt2", [2 * F, BL], dt.bfloat16, kind="ExternalInput").ap()
    w1b1q = nc.dram_tensor("w1b1q", [128, F * H], dt.bfloat16, kind="ExternalInput").ap()
    w2s = nc.dram_tensor("w2s", [H, F * E], dt.bfloat16, kind="ExternalInput").ap()
    outT = nc.dram_tensor("outT", [F * E, BL], dt.bfloat16, kind="ExternalOutput").ap()

    # DRAM views
    # xt2 rows: 2f + r  (f feature, r 0=x / 1=ones); g = 2j + r below
    xt2_r = xt2.rearrange("(q g) n -> g q n", g=8)        # [8, NQUAD, BL]
    outT_r = outT.rearrange("(q p) n -> q p n", p=128)    # [NQUAD, 128, BL]

    for _ in range(CONFIG["VARIANT_ID"]):
        nc.sync.nop()

    relu_act, out_act = CONFIG["RELU_ACT_OF_16"], CONFIG["OUT_ACT_OF_16"]

    with tile.TileContext(nc) as tc:
        with (
            tc.tile_pool(name="params", bufs=1) as params,
            tc.tile_pool(name="h", bufs=4) as h_pool,
            tc.tile_pool(name="outs", bufs=3) as outs_pool,
            tc.tile_pool(name="pre", bufs=3, space="PSUM") as pre_pool,
            tc.tile_pool(name="pout", bufs=2, space="PSUM") as pout_pool,
        ):
            # Two persistent xq buffers (even/odd chunks).  Rows 32j+0/1 of
            # each row group hold (x, ones) per feature.
            xqs = [params.tile([128, NQUAD * CHUNK], dt.bfloat16,
                               tag=f"xq{i}", name=f"xq{i}")
                   for i in range(2)]

            # chunk-0 x data ahead of the params on the sync queue
            xq0 = xqs[0]
            for j in range(4):
                nc.sync.dma_start(
                    out=xq0[32 * j:32 * j + 2, :].rearrange(
                        "r (q n) -> r q n", n=CHUNK),
                    in_=xt2_r[2 * j:2 * j + 2, :, bass.ts(0, CHUNK)],
                )

            w1b1q_sb = params.tile([128, F * H], dt.bfloat16, tag="w1b1q")
            for p in range(4):
                nc.sync.dma_start(out=w1b1q_sb[:, bass.ts(p, F * H // 4)],
                                  in_=w1b1q[:, bass.ts(p, F * H // 4)])
            w2_sb = params.tile([H, F * E], dt.bfloat16, tag="w2s")
            nc.sync.dma_start(out=w2_sb[:], in_=w2s[:])

            relu_credit = 0
            out_credit = 0
            for c in range(NCHUNK):
                xq = xqs[c % 2]
                # xq[32j + r, 512q + cc] = xt2[8q + 2j + r, 512c + cc]
                # (chunk 0's DMAs were issued before the params above)
                if c > 0:
                    for j in range(4):
                        nc.sync.dma_start(
                            out=xq[32 * j:32 * j + 2, :].rearrange(
                                "r (q n) -> r q n", n=CHUNK),
                            in_=xt2_r[2 * j:2 * j + 2, :, bass.ts(c, CHUNK)],
                        )

                hT_prev = None
                for q in range(NQUAD + 1):
                    if q < NQUAD:
                        # ---- L1: 4 features, row groups 0..3, K=2 ----
                        pre_a = pre_pool.tile([128, 2 * CHUNK], dt.float32,
                                              tag="pre")
                        pre_b = pre_pool.tile([128, 2 * CHUNK], dt.float32,
                                              tag="pre")
                        for j in range(4):
                            tgt = pre_a if j < 2 else pre_b
                            nc.tensor.matmul(
                                tgt[:, bass.ts(j % 2, CHUNK)],
                                lhsT=w1b1q_sb[32 * j:32 * j + 2,
                                              bass.ts(q, H)],
                                rhs=xq[32 * j:32 * j + 2, bass.ts(q, CHUNK)],
                                start=True, stop=True,
                                tile_position=(32 * j, 0),
                            )

                        # ---- relu + cast bf16, split ACT / DVE ----
                        hT = h_pool.tile([128, 4 * CHUNK], dt.bfloat16,
                                         tag="h")
                        for half, hsrc in ((0, pre_a), (1, pre_b)):
                            dst = hT[:, bass.ts(half, 2 * CHUNK)]
                            relu_credit += relu_act
                            if relu_credit >= 16:
                                relu_credit -= 16
                                nc.scalar.activation(
                                    dst, hsrc[:],
                                    mybir.ActivationFunctionType.Relu)
                            else:
                                nc.vector.tensor_scalar_max(dst, hsrc[:],
                                                            0.0)
                    if hT_prev is None:
                        hT_prev = hT
                        continue

                    # ---- L2 for the PREVIOUS quad, software-pipelined so
                    # the PE has ready matmuls while this quad's relu runs --
                    qq = q - 1
                    pout = pout_pool.tile([128, CHUNK], dt.float32,
                                          tag="pout")
                    for j in range(4):
                        f = 4 * qq + j
                        nc.tensor.matmul(
                            pout[32 * j:32 * j + 32, :],
                            lhsT=w2_sb[:, bass.ts(f, E)],
                            rhs=hT_prev[:, bass.ts(j, CHUNK)],
                            start=True, stop=True,
                            tile_position=(0, 32 * j),
                        )
                    hT_prev = hT if q < NQUAD else None

                    # ---- copy quad PSUM fp32 -> SBUF bf16 ----
                    if qq % 4 == 0:
                        outTs = outs_pool.tile([128, 4 * CHUNK], dt.bfloat16,
                                               tag="outs")
                    dst = outTs[:, bass.ts(qq % 4, CHUNK)]
                    out_credit += out_act
                    if out_credit >= 16:
                        out_credit -= 16
                        nc.scalar.add(dst, pout[:], 0.0)
                    else:
                        nc.vector.tensor_scalar_add(dst, pout[:], 0.0)

                    # ---- store 4 quads: outT rows 128(qq-3)..128(qq+1) ----
                    # (gpsimd DMA queue, so the serial sync queue stays free
                    # for the next chunk's xq prefetch)
                    if qq % 4 == 3:
                        # SBUF src must keep the partition dim outermost;
                        # permute the DRAM view instead.
                        nc.gpsimd.dma_start(
                            out=outT_r[qq - 3:qq + 1, :, bass.ts(c, CHUNK)
                                       ].rearrange("q p n -> p q n"),
                            in_=outTs[:].rearrange("p (k n) -> p k n",
                                                   n=CHUNK),
                        )

    nc.compile()
    return nc


def _prep_inputs(x, w1, b1, w2, b2):
    """Host-side packing of parameters + per-core x shards."""
    w1b1q = np.zeros((128, F * H), dtype=BF16)
    for f in range(F):
        q, j = divmod(f, 4)
        w1b1q[32 * j + 0, H * q:H * q + H] = w1[f].astype(BF16)
        w1b1q[32 * j + 1, H * q:H * q + H] = b1[f].astype(BF16)

    w2s = np.ascontiguousarray(
        w2.transpose(1, 0, 2).reshape(H, F * E)).astype(BF16)

    in_maps = []
    for core in range(NCORES):
        xs = x[core * BL:(core + 1) * BL]          # [BL, F]
        xt2 = np.empty((2 * F, BL), dtype=BF16)
        xt2[0::2] = xs.T.astype(BF16)
        xt2[1::2] = BF16(1.0)
        in_maps.append({"xt2": xt2, "w1b1q": w1b1q, "w2s": w2s})
    return in_maps


def _get_compiled():
    global _COMPILED
    if _COMPILED is None:
        _COMPILED = _build_bass()
    return _COMPILED


def reset_compiled():
    global _COMPILED
    _COMPILED = None


def kernel(x, w1, b1, w2, b2, _trace=False, _trace_kwargs=None):
    nc = _get_compiled()
    x = np.asarray(x, dtype=np.float32)
    w2 = np.asarray(w2, dtype=np.float32)
    b2 = np.asarray(b2, dtype=np.float32)
    in_maps = _prep_inputs(
        x, np.asarray(w1, dtype=np.float32),
        np.asarray(b1, dtype=np.float32), w2, b2)
    res = run_bass_kernel_spmd(
        nc, in_maps, core_ids=list(range(NCORES)),
        trace=_trace, **(_trace_kwargs or {}))
    b2f = b2.reshape(F * E).astype(np.float32)          # fe = f*E + e
    shards = []
    for i in range(NCORES):
        oT = np.asarray(res.results[i]["outT"])          # [F*E, BL] bf16
        shards.append((oT.astype(np.float32) + b2f[:, None]).T)
    full = np.ascontiguousarray(np.concatenate(shards, axis=0),
                                dtype=np.float32)
    if _trace:
        return full, res
    return full


if __name__ == "__main__":
    rng = np.random.default_rng(0)
    x = rng.standard_normal((B, F), dtype=np.float32)
    w1 = rng.standard_normal((F, H), dtype=np.float32)
    b1 = rng.standard_normal((F, H), dtype=np.float32)
    w2 = (rng.standard_normal((F, H, E), dtype=np.float32) / np.sqrt(H)).astype(np.float32)
    b2 = rng.standard_normal((F, E), dtype=np.float32) / np.sqrt(H)
    got = kernel(x=x, w1=w1, b1=b1, w2=w2, b2=b2)
    h = np.maximum(x[:, :, None] * w1[None] + b1[None], 0.0)
    want = (np.einsum("bfh,fhe->bfe", h, w2) + b2[None]).reshape(B, F * E)
    err = np.abs(got - want).max() / np.abs(want).max()
    print("self-test scale-relative max err:", err)


# revision 24
# speedup vs baseline: 1.0363x; 1.0062x over previous
"""
Trainium2 Bass kernel for nn_DenseFeatureNumericEmbedding (v2).

Computes, per feature f (F=128 independent tiny MLPs):
    h[b,f,:]   = relu(x[b,f] * w1[f,:] + b1[f,:])            # [B, F, H]
    out[b,f,:] = h[b,f,:] @ w2[f,:,:] + b2[f,:]              # [B, F, E]
    returns out.reshape(B, F*E)                              # [16384, 4096] fp32

Sharding: data-parallel over batch across 8 NeuronCores (2048 rows/core),
params replicated. No collectives.

v2 design (vs v1 baseline at 512us):
  - Device stores outT [F*E, BL] in fp16; the HOST transposes to [BL, F*E],
    adds b2, and casts to fp32.  This removes all 512 PE transposes, their
    LDWEIGHTS, the DVE staging copies, and halves output DMA bytes.
  - L1 matmuls are zero-padded to K=32 (stationary rows 2..32 of each row
    group are zero, xq rows 2..32 of each group are memset to zero) so the
    PE array shows high activity and the HAM clock gate stays at 2.4 GHz.
  - PSUM carving: pre pool [128,1024] fp32 x2 bufs (4 banks) for L1 output,
    pout pool [128,1024] fp32 x2 bufs (4 banks) for two quads of L2 output.
    Relu runs at FD=1024 split ACT/DVE; output copy (pure fp32->fp16 copy,
    bias folded out to host) runs at FD=1024.
  - Output DMA batched per 4 quads: 32 DMAs of 512KB, 1KB runs.

Per-core dataflow (per 512-batch chunk, per quad of 4 features):
  L1   TensorE: 4 row-tiled K=32 matmuls -> pre_a/pre_b [128, 1024] fp32.
  RELU ScalarE activation(Relu) / VectorE tensor_scalar_max(0) split,
       PSUM -> SBUF bf16 hT [128, 2048].
  L2   TensorE: 4 col-tiled K=128 matmuls -> pout2 [128, 512] slice.
  COPY fp32 PSUM -> fp16 SBUF staging (DVE tensor_copy / ACT Copy).
  DMA  outT [F*E, BL] fp16, 1KB contiguous runs.
"""

import sys

sys.path.insert(0, "/opt/trn_rl_repo")

import numpy as np
import ml_dtypes

import concourse.bass as bass
import concourse.tile as tile
from concourse import bacc, mybir
from concourse.bass_utils import run_bass_kernel_spmd

BF16 = ml_dtypes.bfloat16
FP16 = np.float16

B = 16384
F = 128
H = 128
E = 32
NCORES = 8
BL = B // NCORES          # 2048 rows per core
CHUNK = 512               # batch columns per inner tile (1 PSUM bank fp32)
NCHUNK = BL // CHUNK      # 4
NQUAD = F // 4            # 32 quads of 4 features

CONFIG = {
    "RELU_ACT_OF_64": 43,  # of every 64 relu instrs, this many on ScalarE
    "OUT_ACT_OF_16": 0,    # of every 16 out-copies, this many on ScalarE
    "VARIANT_ID": 0,       # busts the NEFF cache between variants
}

_COMPILED = None


def _build_bass():
    nc = bacc.Bacc("TRN2", target_bir_lowering=False, debug=False,
                   num_devices=NCORES)
    dt = mybir.dt

    xt2 = nc.dram_tensor("xt2", [2 * F, BL], dt.bfloat16, kind="ExternalInput").ap()
    w1b1q = nc.dram_tensor("w1b1q", [128, F * H], dt.bfloat16, kind="ExternalInput").ap()
    w2s = nc.dram_tensor("w2s", [H, F * E], dt.bfloat16, kind="ExternalInput").ap()
    outT = nc.dram_tensor("outT", [F * E, BL], dt.bfloat16, kind="ExternalOutput").ap()

    # DRAM views
    # xt2 rows: 2f + r  (f feature, r 0=x / 1=ones); g = 2j + r below
    xt2_r = xt2.rearrange("(q g) n -> g q n", g=8)        # [8, NQUAD, BL]
    outT_r = outT.rearrange("(q p) n -> q p n", p=128)    # [NQUAD, 128, BL]

    for _ in range(CONFIG["VARIANT_ID"]):
        nc.sync.nop()

    relu_act, out_act = CONFIG["RELU_ACT_OF_64"], CONFIG["OUT_ACT_OF_16"]

    with tile.TileContext(nc) as tc:
        with (
            tc.tile_pool(name="params", bufs=1) as params,
            tc.tile_pool(name="h", bufs=6) as h_pool,
            tc.tile_pool(name="outs", bufs=4) as outs_pool,
            tc.tile_pool(name="pre", bufs=3, space="PSUM") as pre_pool,
            tc.tile_pool(name="pout", bufs=2, space="PSUM") as pout_pool,
        ):
            # Two persistent xq buffers (even/odd chunks).  Rows 32j+0/1 of
            # each row group hold (x, ones) per feature.
            xqs = [params.tile([128, NQUAD * CHUNK], dt.bfloat16,
                               tag=f"xq{i}", name=f"xq{i}")
                   for i in range(2)]

            # chunk-0 x data ahead of the params on the sync queue
            xq0 = xqs[0]
            for j in range(4):
                nc.sync.dma_start(
                    out=xq0[32 * j:32 * j + 2, :].rearrange(
                        "r (q n) -> r q n", n=CHUNK),
                    in_=xt2_r[2 * j:2 * j + 2, :, bass.ts(0, CHUNK)],
                )

            w1b1q_sb = params.tile([128, F * H], dt.bfloat16, tag="w1b1q")
            for p in range(4):
                nc.sync.dma_start(out=w1b1q_sb[:, bass.ts(p, F * H // 4)],
                                  in_=w1b1q[:, bass.ts(p, F * H // 4)])
            w2_sb = params.tile([H, F * E], dt.bfloat16, tag="w2s")
            nc.sync.dma_start(out=w2_sb[:], in_=w2s[:])

            relu_credit = 0
            out_credit = 0
            # Flattened (chunk, quad) pipeline: L2 lags L1/relu by one unit
            # and the lag carries ACROSS chunk boundaries (no per-chunk
            # pipeline flush).
            units = [(c, q) for c in range(NCHUNK) for q in range(NQUAD)]
            prev = None           # (c, q, hT) of the lagged unit
            for u in range(len(units) + 1):
                if u < len(units):
                    c, q = units[u]
                    xq = xqs[c % 2]
                    # xq[32j + r, 512q + cc] = xt2[8q + 2j + r, 512c + cc]
                    # (chunk 0's DMAs were issued before the params above)
                    if q == 0 and c > 0:
                        for j in range(4):
                            nc.sync.dma_start(
                                out=xq[32 * j:32 * j + 2, :].rearrange(
                                    "r (q n) -> r q n", n=CHUNK),
                                in_=xt2_r[2 * j:2 * j + 2, :,
                                          bass.ts(c, CHUNK)],
                            )

                    # ---- L1: 4 features, row groups 0..3, K=2 ----
                    pre_a = pre_pool.tile([128, 2 * CHUNK], dt.float32,
                                          tag="pre")
                    pre_b = pre_pool.tile([128, 2 * CHUNK], dt.float32,
                                          tag="pre")
                    for j in range(4):
                        tgt = pre_a if j < 2 else pre_b
                        nc.tensor.matmul(
                            tgt[:, bass.ts(j % 2, CHUNK)],
                            lhsT=w1b1q_sb[32 * j:32 * j + 2, bass.ts(q, H)],
                            rhs=xq[32 * j:32 * j + 2, bass.ts(q, CHUNK)],
                            start=True, stop=True,
                            tile_position=(32 * j, 0),
                        )

                    # ---- relu + cast bf16, split ACT / DVE ----
                    hT = h_pool.tile([128, 4 * CHUNK], dt.bfloat16, tag="h")
                    for half, hsrc in ((0, pre_a), (1, pre_b)):
                        dst = hT[:, bass.ts(half, 2 * CHUNK)]
                        relu_credit += relu_act
                        if relu_credit >= 64:
                            relu_credit -= 64
                            nc.scalar.activation(
                                dst, hsrc[:],
                                mybir.ActivationFunctionType.Relu)
                        else:
                            nc.vector.tensor_scalar_max(dst, hsrc[:], 0.0)
                if prev is None:
                    prev = (c, q, hT)
                    continue

                # ---- L2 for the PREVIOUS unit, software-pipelined so the
                # PE has ready matmuls while this unit's relu runs ----
                cc, qq, hT_prev = prev
                prev = (c, q, hT) if u < len(units) else None
                pout = pout_pool.tile([128, CHUNK], dt.float32, tag="pout")
                for j in range(4):
                    f = 4 * qq + j
                    nc.tensor.matmul(
                        pout[32 * j:32 * j + 32, :],
                        lhsT=w2_sb[:, bass.ts(f, E)],
                        rhs=hT_prev[:, bass.ts(j, CHUNK)],
                        start=True, stop=True,
                        tile_position=(0, 32 * j),
                    )

                # ---- copy quad PSUM fp32 -> SBUF bf16 ----
                if qq % 4 == 0:
                    outTs = outs_pool.tile([128, 4 * CHUNK], dt.bfloat16,
                                           tag="outs")
                dst = outTs[:, bass.ts(qq % 4, CHUNK)]
                out_credit += out_act
                if out_credit >= 16:
                    out_credit -= 16
                    nc.scalar.add(dst, pout[:], 0.0)
                else:
                    nc.vector.tensor_scalar_add(dst, pout[:], 0.0)

                # ---- store 4 quads: outT rows 128(qq-3)..128(qq+1) ----
                # (gpsimd DMA queue, so the serial sync queue stays free
                # for the next chunk's xq prefetch)
                if qq % 4 == 3:
                    # SBUF src must keep the partition dim outermost;
                    # permute the DRAM view instead.
                    nc.gpsimd.dma_start(
                        out=outT_r[qq - 3:qq + 1, :, bass.ts(cc, CHUNK)
                                   ].rearrange("q p n -> p q n"),
                        in_=outTs[:].rearrange("p (k n) -> p k n",
                                               n=CHUNK),
                    )

    nc.compile()
    return nc


def _prep_inputs(x, w1, b1, w2, b2):
    """Host-side packing of parameters + per-core x shards."""
    w1b1q = np.zeros((128, F * H), dtype=BF16)
    for f in range(F):
        q, j = divmod(f, 4)
        w1b1q[32 * j + 0, H * q:H * q + H] = w1[f].astype(BF16)
        w1b1q[32 * j + 1, H * q:H * q + H] = b1[f].astype(BF16)

    w2s = np.ascontiguousarray(
        w2.transpose(1, 0, 2).reshape(H, F * E)).astype(BF16)

    in_maps = []
    for core in range(NCORES):
        xs = x[core * BL:(core + 1) * BL]          # [BL, F]
        xt2 = np.empty((2 * F, BL), dtype=BF16)
        xt2[0::2] = xs.T.astype(BF16)
        xt2[1::2] = BF16(1.0)
        in_maps.append({"xt2": xt2, "w1b1q": w1b1q, "w2s": w2s})
    return in_maps


def _get_compiled():
    global _COMPILED
    if _COMPILED is None:
        _COMPILED = _build_bass()
    return _COMPILED


def reset_compiled():
    global _COMPILED
    _COMPILED = None


def kernel(x, w1, b1, w2, b2, _trace=False, _trace_kwargs=None):
    nc = _get_compiled()
    x = np.asarray(x, dtype=np.float32)
    w2 = np.asarray(w2, dtype=np.float32)
    b2 = np.asarray(b2, dtype=np.float32)
    in_maps = _prep_inputs(
        x, np.asarray(w1, dtype=np.float32),
        np.asarray(b1, dtype=np.float32), w2, b2)
    res = run_bass_kernel_spmd(
        nc, in_maps, core_ids=list(range(NCORES)),
        trace=_trace, **(_trace_kwargs or {}))
    b2f = b2.reshape(F * E).astype(np.float32)          # fe = f*E + e
    shards = []
    for i in range(NCORES):
        oT = np.asarray(res.results[i]["outT"])          # [F*E, BL] bf16
        shards.append((oT.astype(np.float32) + b2f[:, None]).T)
    full = np.ascontiguousarray(np.concatenate(shards, axis=0),
                                dtype=np.float32)
    if _trace:
        return full, res
    return full


if __name__ == "__main__":
    rng = np.random.default_rng(0)
    x = rng.standard_normal((B, F), dtype=np.float32)
    w1 = rng.standard_normal((F, H), dtype=np.float32)
    b1 = rng.standard_normal((F, H), dtype=np.float32)
    w2 = (rng.standard_normal((F, H, E), dtype=np.float32) / np.sqrt(H)).astype(np.float32)
    b2 = rng.standard_normal((F, E), dtype=np.float32) / np.sqrt(H)
    got = kernel(x=x, w1=w1, b1=b1, w2=w2, b2=b2)
    h = np.maximum(x[:, :, None] * w1[None] + b1[None], 0.0)
    want = (np.einsum("bfh,fhe->bfe", h, w2) + b2[None]).reshape(B, F * E)
    err = np.abs(got - want).max() / np.abs(want).max()
    print("self-test scale-relative max err:", err)
